# revision 5
# baseline (speedup 1.0000x reference)
"""Trainium2 Bass kernel for nn_BilinearUpsampler.

out[b,c,i,j] = sum_k softmax_k(MLP(poseMap)[c,k,i,j]) * xpad[b,c,Y[i,j]+dy_k,X[i,j]+dx_k]

Strategy (8 NeuronCores, output-pixel sharded, 32768 px/core):
  - pixels-on-partitions layout: every on-chip tensor is [128 pixels, ...free]
  - MLP (3->64->256->576 1x1 convs) on PE; final matmul flipped (lhsT = h2
    tile) so logits land as [128 px, 576] in PSUM; bias b3 added via a K=1
    ones-row matmul; exp eviction on ACT -> e [128, 576] bf16
  - 3x3 patch gather via gpsimd.dma_gather from DRAM: x stored as
    [130*130 cells, 128 (b,c)] bf16; one 768B descriptor per (pixel, dy)
    with elem_step=128 elems (256B) giving overlapping 3-cell windows
  - 9-tap weighted sum + softmax denominator + normalize on DVE (bf16 2x)
  - per-core output [32768 px, 128 (b,c)] f32; host reassembles layout
"""

import sys
import os

sys.path.insert(0, "/opt/trn_rl_repo")

import numpy as np
import ml_dtypes

import concourse.bass as bass
import concourse.bacc as bacc
import concourse.mybir as mybir
import concourse.tile as tile
from concourse.bass_utils import run_bass_kernel_spmd
import bass_rust

BF16 = mybir.dt.bfloat16
F32 = mybir.dt.float32
I16 = mybir.dt.int16
AF = mybir.ActivationFunctionType

NCORES = 8
C = 64
KS = 3
BS = 2
HI = WI = 128
HO = WO = 512
HP = HI + 2  # 130 padded
NCELL = HP * HP  # 16900
NWIN = NCELL - 2  # gatherable 3-cell windows
PXTOT = HO * WO
PX = PXTOT // NCORES  # 32768 pixels per core

TT = 1024  # pixel tile
SUB = TT // 128  # 8 subtiles of 128 px
NT = PX // TT  # 32 tiles

LAST_RESULT = None  # BassKernelResults of the most recent run (for test.py)

_PROG_CACHE = {}


def build_program(px=PX, tt=TT):
    sub = tt // 128
    nt = px // tt
    nc = bacc.Bacc("TRN2", target_bir_lowering=False, debug=False,
                   num_devices=NCORES)

    xw_d = nc.dram_tensor("xw", [NCELL * 128], BF16, kind="ExternalInput")
    idx_d = nc.dram_tensor("idxw", [3, 128, px // 16], I16, kind="ExternalInput")
    pose_d = nc.dram_tensor("pose", [3, px], BF16, kind="ExternalInput")
    w1t_d = nc.dram_tensor("w1t", [3, 64], BF16, kind="ExternalInput")
    w2t_d = nc.dram_tensor("w2t", [64, 256], BF16, kind="ExternalInput")
    w3km_d = nc.dram_tensor("w3km", [256, 576], BF16, kind="ExternalInput")
    b3km_d = nc.dram_tensor("b3km", [1, 576], BF16, kind="ExternalInput")
    b1_d = nc.dram_tensor("b1", [64, 1], F32, kind="ExternalInput")
    b2_d = nc.dram_tensor("b2", [256, 1], F32, kind="ExternalInput")
    out_d = nc.dram_tensor("out", [nt, sub, 128, 128], BF16, kind="ExternalOutput")

    # overlapping 3-cell window view of x: [NWIN, 384] with row stride 128
    def x_windows_ap():
        ap = xw_d[:].copy()
        ap.ap = bass_rust.VecI64Pair([(128, NWIN), (1, 384)])
        return ap

    with tile.TileContext(nc) as tc:
        with (
            tc.tile_pool(name="consts", bufs=1) as cpool,
            tc.tile_pool(name="mlp", bufs=2) as mpool,
            tc.tile_pool(name="gath", bufs=2) as gpool,
            tc.tile_pool(name="ework", bufs=2) as epool,
            tc.tile_pool(name="dve", bufs=1) as vpool,
            tc.tile_pool(name="outp", bufs=2) as opool,
            tc.tile_pool(name="ph1", bufs=1, space="PSUM") as ph1,
            tc.tile_pool(name="ph2", bufs=2, space="PSUM") as ph2,
            tc.tile_pool(name="pw", bufs=2, space="PSUM") as pw,
        ):
            # ---- constants ----
            w1t = cpool.tile([3, 64], BF16, tag="w1t")
            nc.sync.dma_start(w1t[:], w1t_d[:])
            w2t = cpool.tile([64, 256], BF16, tag="w2t")
            nc.sync.dma_start(w2t[:], w2t_d[:])
            w3km0 = cpool.tile([128, 576], BF16, tag="w3km0")
            nc.sync.dma_start(w3km0[:], w3km_d[0:128])
            w3km1 = cpool.tile([128, 576], BF16, tag="w3km1")
            nc.sync.dma_start(w3km1[:], w3km_d[128:256])
            b3km = cpool.tile([1, 576], BF16, tag="b3km")
            nc.sync.dma_start(b3km[:], b3km_d[:])
            b1t = cpool.tile([64, 1], F32, tag="b1t")
            nc.sync.dma_start(b1t[:], b1_d[:])
            b2t0 = cpool.tile([128, 1], F32, tag="b2t0")
            nc.sync.dma_start(b2t0[:], b2_d[0:128])
            b2t1 = cpool.tile([128, 1], F32, tag="b2t1")
            nc.sync.dma_start(b2t1[:], b2_d[128:256])
            ones = cpool.tile([1, 128], BF16, tag="ones")
            nc.vector.memset(ones[:], 1.0)
            idxt = cpool.tile([128, 3, px // 16], I16, tag="idxt")
            for dy in range(3):
                nc.sync.dma_start(idxt[:, dy, :], idx_d[dy])

            xwin = x_windows_ap()

            for t in range(nt):
                # ---- MLP stage ----
                p3 = mpool.tile([3, tt], BF16, tag="p3")
                nc.sync.dma_start(p3[:], pose_d[:, t * tt:(t + 1) * tt])
                h1s = mpool.tile([64, tt], BF16, tag="h1s")
                h2s0 = mpool.tile([128, tt], BF16, tag="h2s0")
                h2s1 = mpool.tile([128, tt], BF16, tag="h2s1")
                for q in range(tt // 512):
                    qs = slice(q * 512, (q + 1) * 512)
                    h1p = ph1.tile([64, 512], F32, tag="h1p")
                    nc.tensor.matmul(h1p[:], w1t[:], p3[:, qs],
                                     start=True, stop=True)
                    nc.scalar.activation(h1s[:, qs], h1p[:], AF.Relu,
                                         bias=b1t[:])
                    for cc, (h2s, b2t) in ((0, (h2s0, b2t0)), (1, (h2s1, b2t1))):
                        h2p = ph2.tile([128, 512], F32, tag="h2p")
                        nc.tensor.matmul(h2p[:], w2t[:, cc * 128:(cc + 1) * 128],
                                         h1s[:, qs], start=True, stop=True)
                        nc.scalar.activation(h2s[:, qs], h2p[:], AF.Relu,
                                             bias=b2t[:])

                # ---- gather stage ----
                g = gpool.tile([128, 3, sub, 384], BF16, tag="g")
                for dy in range(3):
                    nc.gpsimd.dma_gather(
                        out_ap=g[:, dy, :, :],
                        in_ap=xwin,
                        idxs_ap=idxt[:, dy, t * (tt // 16):(t + 1) * (tt // 16)],
                        num_idxs=tt,
                        num_idxs_reg=tt,
                        elem_size=384,
                        elem_step=128,
                    )

                # ---- logits + exp ----
                e_t = epool.tile([128, sub, 576], BF16, tag="e_t")
                for s in range(sub):
                    ts = slice(t * tt + s * 128 - t * tt, s * 128 + 128)
                    ss = slice(s * 128, s * 128 + 128)
                    wp = pw.tile([128, 576], F32, tag="wp")
                    for r0, r1 in ((0, 512), (512, 576)):
                        nc.tensor.matmul(wp[:, r0:r1], h2s0[:, ss],
                                         w3km0[:, r0:r1], start=True, stop=False)
                        nc.tensor.matmul(wp[:, r0:r1], h2s1[:, ss],
                                         w3km1[:, r0:r1], start=False, stop=False)
                        nc.tensor.matmul(wp[:, r0:r1], ones[:],
                                         b3km[:, r0:r1], start=False, stop=True)
                    nc.scalar.activation(e_t[:, s, :], wp[:], AF.Exp)

                # ---- taps on DVE ----
                prods = vpool.tile([128, 9, sub, 128], BF16, tag="prods")
                for k in range(9):
                    dy, dx = k // 3, k % 3
                    g_k = g[:, dy, :, dx * 128:(dx + 1) * 128].rearrange(
                        "p s (b c) -> p s b c", b=2)
                    e_k = e_t[:, :, k * 64:(k + 1) * 64].unsqueeze(2)
                    e_k = e_k.broadcast_to((128, sub, 2, 64))
                    pk = prods[:, k, :, :].rearrange("p s (b c) -> p s b c", b=2)
                    nc.vector.tensor_mul(pk, g_k, e_k)
                q1 = vpool.tile([128, 4, sub, 128], BF16, tag="q1")
                nc.vector.tensor_add(q1[:], prods[:, 0:4], prods[:, 4:8])
                q2 = vpool.tile([128, 2, sub, 128], BF16, tag="q2")
                nc.vector.tensor_add(q2[:], q1[:, 0:2], q1[:, 2:4])
                acc = vpool.tile([128, sub, 128], BF16, tag="acc")
                nc.vector.tensor_add(acc[:], q2[:, 0], q2[:, 1])
                acc2 = vpool.tile([128, sub, 128], BF16, tag="acc2")
                nc.vector.tensor_add(acc2[:], acc[:], prods[:, 8])

                # ---- softmax denominator ----
                d1 = vpool.tile([128, sub, 256], BF16, tag="d1")
                nc.vector.tensor_add(d1[:], e_t[:, :, 0:256], e_t[:, :, 256:512])
                d2 = vpool.tile([128, sub, 128], F32, tag="d2")
                nc.vector.tensor_add(d2[:], d1[:, :, 0:128], d1[:, :, 128:256])
                d3 = vpool.tile([128, sub, 64], F32, tag="d3")
                nc.vector.tensor_add(d3[:], d2[:, :, 0:64], d2[:, :, 64:128])
                den = vpool.tile([128, sub, 64], F32, tag="den")
                nc.vector.tensor_add(den[:], d3[:], e_t[:, :, 512:576])
                rden = vpool.tile([128, sub, 64], F32, tag="rden")
                nc.vector.reciprocal(rden[:], den[:])

                # ---- normalize + store ----
                out_t = opool.tile([128, sub, 128], BF16, tag="out_t")
                ov = out_t[:].rearrange("p s (b c) -> p s b c", b=2)
                av = acc2[:].rearrange("p s (b c) -> p s b c", b=2)
                rv = rden[:].unsqueeze(2).broadcast_to((128, sub, 2, 64))
                nc.vector.tensor_mul(ov, av, rv)
                dview = out_d[t].rearrange("s p b -> p s b")
                nc.sync.dma_start(dview, out_t[:])

    nc.compile()
    return nc


def _host_prep(x, poseMap, W1, b1, W2, b2, W3, b3, interMapY, interMapX,
               px=PX, tt=TT):
    bf = ml_dtypes.bfloat16
    xp = np.pad(np.asarray(x, np.float32), ((0, 0), (0, 0), (1, 1), (1, 1)))
    # [cell, (b,c)] with c minor
    xw = np.ascontiguousarray(np.transpose(xp, (2, 3, 0, 1))).reshape(NCELL * 128)
    xw = xw.astype(bf)

    Y = np.asarray(interMapY).astype(np.int64).reshape(-1)
    X = np.asarray(interMapX).astype(np.int64).reshape(-1)
    m = ((Y + 0) * HP + X).astype(np.int32)  # base window (padded coords)

    pose = np.asarray(poseMap, np.float32)[0].reshape(3, PXTOT)

    w1t = np.ascontiguousarray(np.asarray(W1, np.float32).T).astype(bf)  # [3,64]
    w2t = np.ascontiguousarray(np.asarray(W2, np.float32).T).astype(bf)  # [64,256]
    W3r = np.asarray(W3, np.float32).reshape(C, 9, 256)
    w3km = np.ascontiguousarray(np.transpose(W3r, (2, 1, 0))).reshape(256, 576)
    w3km = w3km.astype(bf)
    b3km = np.ascontiguousarray(
        np.asarray(b3, np.float32).reshape(C, 9).T).reshape(1, 576).astype(bf)
    b1c = np.asarray(b1, np.float32).reshape(64, 1)
    b2c = np.asarray(b2, np.float32).reshape(256, 1)

    in_maps = []
    for core in range(NCORES):
        sl = slice(core * px, (core + 1) * px)
        mc = m[sl]
        idxw = np.empty((3, 128, px // 16), np.int16)
        for dy in range(3):
            a = (mc + HP * dy).astype(np.int16).reshape(px // 16, 16)
            idxw[dy] = np.tile(a.T, (8, 1))
        in_maps.append({
            "xw": xw,
            "idxw": idxw,
            "pose": np.ascontiguousarray(pose[:, sl]).astype(bf),
            "w1t": w1t, "w2t": w2t, "w3km": w3km, "b3km": b3km,
            "b1": b1c, "b2": b2c,
        })
    return in_maps


def kernel(**inputs):
    global LAST_RESULT
    key = (PX, TT)
    if key not in _PROG_CACHE:
        _PROG_CACHE[key] = build_program(PX, TT)
    nc = _PROG_CACHE[key]
    in_maps = _host_prep(**inputs)
    os.environ.setdefault("BASS_NEVER_TRACE", "1")
    res = run_bass_kernel_spmd(nc, in_maps, list(range(NCORES)))
    LAST_RESULT = res
    parts = [np.asarray(r["out"]).reshape(PX, 128) for r in res.results]
    full = np.concatenate(parts, axis=0).astype(np.float32)  # [PXTOT, 128]
    out = full.reshape(HO, WO, BS, C).transpose(2, 3, 0, 1)
    return np.ascontiguousarray(out)


if __name__ == "__main__":
    import json
    data = np.load(sys.argv[1] if len(sys.argv) > 1 else "work/inputs.npz")
    out = kernel(**{k: data[k] for k in data.files})
    print("out", out.shape, out.dtype, float(np.abs(out).max()))


# revision 7
# speedup vs baseline: 1.2066x; 1.2066x over previous
"""Trainium2 Bass kernel for nn_BilinearUpsampler.

out[b,c,i,j] = sum_k softmax_k(MLP(poseMap)[c,k,i,j]) * xpad[b,c,Y[i,j]+dy_k,X[i,j]+dx_k]

Strategy (8 NeuronCores, output-pixel sharded, 32768 px/core):
  - pixels-on-partitions layout: every on-chip tensor is [128 pixels, ...free]
  - MLP (3->64->256->576 1x1 convs) on PE; final matmul flipped (lhsT = h2
    tile) so logits land as [128 px, 576] in PSUM; bias b3 added via a K=1
    ones-row matmul; exp eviction on ACT -> e [128, 576] bf16
  - 3x3 patch gather via gpsimd.dma_gather from DRAM: x stored as
    [130*130 cells, 128 (b,c)] bf16; one 768B descriptor per (pixel, dy)
    with elem_step=128 elems (256B) giving overlapping 3-cell windows
  - 9-tap weighted sum + softmax denominator + normalize on DVE (bf16 2x)
  - per-core output [32768 px, 128 (b,c)] f32; host reassembles layout
"""

import sys
import os

sys.path.insert(0, "/opt/trn_rl_repo")

import numpy as np
import ml_dtypes

import concourse.bass as bass
import concourse.bacc as bacc
import concourse.mybir as mybir
import concourse.tile as tile
from concourse.bass_utils import run_bass_kernel_spmd
import bass_rust

BF16 = mybir.dt.bfloat16
F32 = mybir.dt.float32
I16 = mybir.dt.int16
AF = mybir.ActivationFunctionType

NCORES = 8
C = 64
KS = 3
BS = 2
HI = WI = 128
HO = WO = 512
HP = HI + 2  # 130 padded
NCELL = HP * HP  # 16900
NWIN = NCELL - 2  # gatherable 3-cell windows
PXTOT = HO * WO
PX = PXTOT // NCORES  # 32768 pixels per core

TT = 1024  # pixel tile
SUB = TT // 128  # 8 subtiles of 128 px
NT = PX // TT  # 32 tiles

LAST_RESULT = None  # BassKernelResults of the most recent run (for test.py)

_PROG_CACHE = {}


def build_program(px=PX, tt=TT):
    sub = tt // 128
    nt = px // tt
    nc = bacc.Bacc("TRN2", target_bir_lowering=False, debug=False,
                   num_devices=NCORES)

    xw_d = nc.dram_tensor("xw", [NCELL * 128], BF16, kind="ExternalInput")
    idx_d = nc.dram_tensor("idxw", [3, 128, px // 16], I16, kind="ExternalInput")
    pose_d = nc.dram_tensor("pose", [3, px], BF16, kind="ExternalInput")
    w1t_d = nc.dram_tensor("w1t", [3, 64], BF16, kind="ExternalInput")
    w2t_d = nc.dram_tensor("w2t", [64, 256], BF16, kind="ExternalInput")
    w3km_d = nc.dram_tensor("w3km", [256, 576], BF16, kind="ExternalInput")
    b3km_d = nc.dram_tensor("b3km", [1, 576], BF16, kind="ExternalInput")
    b1_d = nc.dram_tensor("b1", [64, 1], F32, kind="ExternalInput")
    b2_d = nc.dram_tensor("b2", [256, 1], F32, kind="ExternalInput")
    out_d = nc.dram_tensor("out", [nt, sub, 128, 128], BF16, kind="ExternalOutput")

    # overlapping 3-cell window view of x: [NWIN, 384] with row stride 128
    def x_windows_ap():
        ap = xw_d[:].copy()
        ap.ap = bass_rust.VecI64Pair([(128, NWIN), (1, 384)])
        return ap

    with tile.TileContext(nc) as tc:
        with (
            tc.tile_pool(name="consts", bufs=1) as cpool,
            tc.tile_pool(name="mlp", bufs=2) as mpool,
            tc.tile_pool(name="gath", bufs=2) as gpool,
            tc.tile_pool(name="ework", bufs=2) as epool,
            tc.tile_pool(name="dve", bufs=1) as vpool,
            tc.tile_pool(name="outp", bufs=2) as opool,
            tc.tile_pool(name="ph1", bufs=1, space="PSUM") as ph1,
            tc.tile_pool(name="ph2", bufs=2, space="PSUM") as ph2,
            tc.tile_pool(name="pw", bufs=2, space="PSUM") as pw,
        ):
            # ---- constants ----
            w1t = cpool.tile([3, 64], BF16, tag="w1t")
            nc.sync.dma_start(w1t[:], w1t_d[:])
            w2t = cpool.tile([64, 256], BF16, tag="w2t")
            nc.sync.dma_start(w2t[:], w2t_d[:])
            w3km0 = cpool.tile([128, 576], BF16, tag="w3km0")
            nc.sync.dma_start(w3km0[:], w3km_d[0:128])
            w3km1 = cpool.tile([128, 576], BF16, tag="w3km1")
            nc.sync.dma_start(w3km1[:], w3km_d[128:256])
            b3km = cpool.tile([1, 576], BF16, tag="b3km")
            nc.sync.dma_start(b3km[:], b3km_d[:])
            b1t = cpool.tile([64, 1], F32, tag="b1t")
            nc.sync.dma_start(b1t[:], b1_d[:])
            b2t0 = cpool.tile([128, 1], F32, tag="b2t0")
            nc.sync.dma_start(b2t0[:], b2_d[0:128])
            b2t1 = cpool.tile([128, 1], F32, tag="b2t1")
            nc.sync.dma_start(b2t1[:], b2_d[128:256])
            ones = cpool.tile([1, 128], BF16, tag="ones")
            nc.vector.memset(ones[:], 1.0)
            idxt = cpool.tile([128, 3, px // 16], I16, tag="idxt")
            for dy in range(3):
                nc.sync.dma_start(idxt[:, dy, :], idx_d[dy])

            xwin = x_windows_ap()

            for t in range(nt):
                # ---- MLP stage ----
                p3 = mpool.tile([3, tt], BF16, tag="p3")
                nc.sync.dma_start(p3[:], pose_d[:, t * tt:(t + 1) * tt])
                h1s = mpool.tile([64, tt], BF16, tag="h1s")
                h2s0 = mpool.tile([128, tt], BF16, tag="h2s0")
                h2s1 = mpool.tile([128, tt], BF16, tag="h2s1")
                for q in range(tt // 512):
                    qs = slice(q * 512, (q + 1) * 512)
                    h1p = ph1.tile([64, 512], F32, tag="h1p")
                    nc.tensor.matmul(h1p[:], w1t[:], p3[:, qs],
                                     start=True, stop=True)
                    nc.scalar.activation(h1s[:, qs], h1p[:], AF.Relu,
                                         bias=b1t[:])
                    for cc, (h2s, b2t) in ((0, (h2s0, b2t0)), (1, (h2s1, b2t1))):
                        h2p = ph2.tile([128, 512], F32, tag="h2p")
                        nc.tensor.matmul(h2p[:], w2t[:, cc * 128:(cc + 1) * 128],
                                         h1s[:, qs], start=True, stop=True)
                        nc.scalar.activation(h2s[:, qs], h2p[:], AF.Relu,
                                             bias=b2t[:])

                # ---- gather stage ----
                g = gpool.tile([128, 3, sub, 384], BF16, tag="g")
                for dy in range(3):
                    nc.gpsimd.dma_gather(
                        out_ap=g[:, dy, :, :],
                        in_ap=xwin,
                        idxs_ap=idxt[:, dy, t * (tt // 16):(t + 1) * (tt // 16)],
                        num_idxs=tt,
                        num_idxs_reg=tt,
                        elem_size=384,
                        elem_step=128,
                    )

                # ---- logits + exp ----
                e_t = epool.tile([128, sub, 576], BF16, tag="e_t")
                for s in range(sub):
                    ts = slice(t * tt + s * 128 - t * tt, s * 128 + 128)
                    ss = slice(s * 128, s * 128 + 128)
                    wp = pw.tile([128, 576], F32, tag="wp")
                    for r0, r1 in ((0, 512), (512, 576)):
                        nc.tensor.matmul(wp[:, r0:r1], h2s0[:, ss],
                                         w3km0[:, r0:r1], start=True, stop=False)
                        nc.tensor.matmul(wp[:, r0:r1], h2s1[:, ss],
                                         w3km1[:, r0:r1], start=False, stop=False)
                        nc.tensor.matmul(wp[:, r0:r1], ones[:],
                                         b3km[:, r0:r1], start=False, stop=True)
                    nc.scalar.activation(e_t[:, s, :], wp[:], AF.Exp)

                # ---- taps on DVE ----
                prods = vpool.tile([128, 9, sub, 128], BF16, tag="prods")
                for k in range(9):
                    dy, dx = k // 3, k % 3
                    g_k = g[:, dy, :, dx * 128:(dx + 1) * 128].rearrange(
                        "p s (b c) -> p s b c", b=2)
                    e_k = e_t[:, :, k * 64:(k + 1) * 64].unsqueeze(2)
                    e_k = e_k.broadcast_to((128, sub, 2, 64))
                    pk = prods[:, k, :, :].rearrange("p s (b c) -> p s b c", b=2)
                    nc.vector.tensor_mul(pk, g_k, e_k)
                q1 = vpool.tile([128, 4, sub, 128], BF16, tag="q1")
                nc.vector.tensor_add(q1[:], prods[:, 0:4], prods[:, 4:8])
                q2 = vpool.tile([128, 2, sub, 128], BF16, tag="q2")
                nc.vector.tensor_add(q2[:], q1[:, 0:2], q1[:, 2:4])
                acc = vpool.tile([128, sub, 128], BF16, tag="acc")
                nc.vector.tensor_add(acc[:], q2[:, 0], q2[:, 1])
                acc2 = vpool.tile([128, sub, 128], BF16, tag="acc2")
                nc.vector.tensor_add(acc2[:], acc[:], prods[:, 8])

                # ---- softmax denominator ----
                d1 = vpool.tile([128, sub, 256], BF16, tag="d1")
                nc.vector.tensor_add(d1[:], e_t[:, :, 0:256], e_t[:, :, 256:512])
                d2 = vpool.tile([128, sub, 128], F32, tag="d2")
                nc.gpsimd.tensor_add(d2[:], d1[:, :, 0:128], d1[:, :, 128:256])
                d3 = vpool.tile([128, sub, 64], F32, tag="d3")
                nc.gpsimd.tensor_add(d3[:], d2[:, :, 0:64], d2[:, :, 64:128])
                den = vpool.tile([128, sub, 64], F32, tag="den")
                nc.gpsimd.tensor_add(den[:], d3[:], e_t[:, :, 512:576])
                rden = vpool.tile([128, sub, 64], F32, tag="rden")
                nc.vector.reciprocal(rden[:], den[:])

                # ---- normalize + store ----
                out_t = opool.tile([128, sub, 128], BF16, tag="out_t")
                ov = out_t[:].rearrange("p s (b c) -> p s b c", b=2)
                av = acc2[:].rearrange("p s (b c) -> p s b c", b=2)
                rv = rden[:].unsqueeze(2).broadcast_to((128, sub, 2, 64))
                nc.gpsimd.tensor_mul(ov, av, rv)
                dview = out_d[t].rearrange("s p b -> p s b")
                nc.sync.dma_start(dview, out_t[:])

    nc.compile()
    return nc


def _host_prep(x, poseMap, W1, b1, W2, b2, W3, b3, interMapY, interMapX,
               px=PX, tt=TT):
    bf = ml_dtypes.bfloat16
    xp = np.pad(np.asarray(x, np.float32), ((0, 0), (0, 0), (1, 1), (1, 1)))
    # [cell, (b,c)] with c minor
    xw = np.ascontiguousarray(np.transpose(xp, (2, 3, 0, 1))).reshape(NCELL * 128)
    xw = xw.astype(bf)

    Y = np.asarray(interMapY).astype(np.int64).reshape(-1)
    X = np.asarray(interMapX).astype(np.int64).reshape(-1)
    m = ((Y + 0) * HP + X).astype(np.int32)  # base window (padded coords)

    pose = np.asarray(poseMap, np.float32)[0].reshape(3, PXTOT)

    w1t = np.ascontiguousarray(np.asarray(W1, np.float32).T).astype(bf)  # [3,64]
    w2t = np.ascontiguousarray(np.asarray(W2, np.float32).T).astype(bf)  # [64,256]
    W3r = np.asarray(W3, np.float32).reshape(C, 9, 256)
    w3km = np.ascontiguousarray(np.transpose(W3r, (2, 1, 0))).reshape(256, 576)
    w3km = w3km.astype(bf)
    b3km = np.ascontiguousarray(
        np.asarray(b3, np.float32).reshape(C, 9).T).reshape(1, 576).astype(bf)
    b1c = np.asarray(b1, np.float32).reshape(64, 1)
    b2c = np.asarray(b2, np.float32).reshape(256, 1)

    in_maps = []
    for core in range(NCORES):
        sl = slice(core * px, (core + 1) * px)
        mc = m[sl]
        idxw = np.empty((3, 128, px // 16), np.int16)
        for dy in range(3):
            a = (mc + HP * dy).astype(np.int16).reshape(px // 16, 16)
            idxw[dy] = np.tile(a.T, (8, 1))
        in_maps.append({
            "xw": xw,
            "idxw": idxw,
            "pose": np.ascontiguousarray(pose[:, sl]).astype(bf),
            "w1t": w1t, "w2t": w2t, "w3km": w3km, "b3km": b3km,
            "b1": b1c, "b2": b2c,
        })
    return in_maps


def kernel(**inputs):
    global LAST_RESULT
    key = (PX, TT)
    if key not in _PROG_CACHE:
        _PROG_CACHE[key] = build_program(PX, TT)
    nc = _PROG_CACHE[key]
    in_maps = _host_prep(**inputs)
    os.environ.setdefault("BASS_NEVER_TRACE", "1")
    res = run_bass_kernel_spmd(nc, in_maps, list(range(NCORES)))
    LAST_RESULT = res
    parts = [np.asarray(r["out"]).reshape(PX, 128) for r in res.results]
    full = np.concatenate(parts, axis=0).astype(np.float32)  # [PXTOT, 128]
    out = full.reshape(HO, WO, BS, C).transpose(2, 3, 0, 1)
    return np.ascontiguousarray(out)


if __name__ == "__main__":
    import json
    data = np.load(sys.argv[1] if len(sys.argv) > 1 else "work/inputs.npz")
    out = kernel(**{k: data[k] for k in data.files})
    print("out", out.shape, out.dtype, float(np.abs(out).max()))
